# revision 44
# baseline (speedup 1.0000x reference)
"""Trainium2 Bass kernel for nn_GatedBlock (moe_routing).

Math (reference collapses): the (NB,BS,BS) reshape of weight maps block k to
rows [128k, 128k+128) of weight, so
    out[b, i] = g[b, i // 128] * (x @ W.T)[b, i] + bias[i]
with g = sigmoid(x @ gate_w + gate_b), bottom-8 of 16 gates zeroed per row.

Sharding: output-dim (i) split 8 ways -> 256 rows of W (= 2 gate blocks) per
core.  Per-core inputs (k-tile-major, partition-contiguous rows):
  small (128, KT, 96) bf16  [x_hi | x_lo | gw_hi | gw_lo]
  rhs   (128, KT, 256) bf16 W_shard.T
  epi   (32, 16) f32        gate_b[perm] broadcast over batch
  epib  (128, 2) f32        bias per output block (partition = i)

Design notes (from trace analysis):
* Main matmul runs W-STATIONARY (lhsT = W k-tile [128,128], moving = x_hi
  [128,32] -> psum holds out.T).  LDWEIGHTS ingests the stationary at ~4
  cols/cycle, so a (LDW, MM) pair takes ~27ns vs ~213ns for the x-stationary
  form — W enters the PE 4x faster.  Output leaves transposed; the host
  un-transposes (32KB, trivial).
* Gate logits use an exact bf16 hi/lo split (x@gw to ~1e-5; bf16 products
  are exact, fp32 PSUM accumulate, only the x_lo*gw_lo term is dropped).
  Top-8 RANKING is done on these logits (monotonicity of sigmoid); plain
  bf16 would flip the selection (min margin 3.4e-4) which is catastrophic.
* Sigmoid VALUE comes from a degree-13 odd polynomial on the DVE (3e-4 abs
  err on the logit range).  This keeps the scalar engine activation-free:
  ACT_TABLE_LOADs were observed to stall the scalar HWDGE queue ~1.4us.
* Gating in the transposed orientation: gk rows are partition-broadcast via
  K=1 ones-matmuls (DVE 32x32 block-transposes put each block's gate row at
  partition 0 first).  Epilogue block 0 runs on DVE, block 1 on GpSimd.
* W in bf16 halves the dominant DMA (1.9e-3 rel err vs the 2e-2 gate).  The
  two HWDGE queues sustain ~130-150GB/s each concurrently; bytes are split
  so both queues finish together, with W arrival order matching the main
  matmuls' k-order consumption.
"""

import sys

for _p in ("/opt/trn_rl_repo", "/root/.axon_site/_ro/trn_rl_repo"):
    if _p not in sys.path:
        sys.path.append(_p)

import numpy as np

B = 32          # batch
D = 2048        # model dim
NB = 16         # gate blocks
BLK = D // NB   # 128 output rows per gate block
N_CORES = 8
NOUT = D // N_CORES       # 256 output cols per core
KT = D // 128             # 16 k-tiles
KTG = KT + 1              # gate k-tiles: +1 tile carrying the gate bias
NSM = 96                  # small cols: 64 x (hi|lo) + 32 gw (hi|lo)

# DMA plan: per queue (sync=0, scalar=1), ordered entries.  The scalar
# queue's start is taxed by the sigmoid ACT_TABLE fetches, so it carries
# only W; the k-ranges are ordered to match the main matmuls' consumption.
DEFAULT_PLAN = {
    0: [("epib",), ("small", 0, 8), ("rhs", 8, 12), ("rhs", 12, 16)],
    1: [("small", 8, KTG), ("rhs", 0, 4), ("rhs", 4, 8)],
}

_compiled = {}


def _build(plan):
    import concourse.bacc as bacc
    import concourse.tile as tile
    import concourse.mybir as mybir

    f32 = mybir.dt.float32
    bf16 = mybir.dt.bfloat16
    Alu = mybir.AluOpType

    nc = bacc.Bacc("TRN2", target_bir_lowering=False, debug=False,
                   num_devices=N_CORES)

    small_d = nc.dram_tensor("small", [128, KTG, NSM], bf16, kind="ExternalInput")
    rhs_d = nc.dram_tensor("rhs", [128, KT, NOUT], bf16, kind="ExternalInput")
    epib_d = nc.dram_tensor("epib", [BLK, 2], f32, kind="ExternalInput")
    out_d = nc.dram_tensor("out", [BLK, 2 * B], f32, kind="ExternalOutput")

    with tile.TileContext(nc) as tc:
        with (
            tc.tile_pool(name="sb", bufs=1) as sb,
            tc.tile_pool(name="ps", bufs=1, space="PSUM") as psp,
        ):
            small = sb.tile([128, KTG, NSM], bf16, name="small_sb", tag="small_sb")
            rhs = sb.tile([128, KT, NOUT], bf16, name="rhs_sb", tag="rhs_sb")
            epib = sb.tile([BLK, 2], f32, name="epib_sb", tag="epib_sb")
            t1 = sb.tile([B, NB], f32, name="t1", tag="t1")
            graw = sb.tile([B, NB], f32, name="graw", tag="graw")
            g = sb.tile([B, NB], f32, name="g", tag="g")
            m8 = sb.tile([B, 8], f32, name="m8", tag="m8")
            rep = sb.tile([B, NB], f32, name="rep", tag="rep")
            gk = sb.tile([B, NB], f32, name="gk", tag="gk")
            ones = sb.tile([1, BLK], bf16, name="ones", tag="ones")
            gkp2 = sb.tile([B, 2 * B], bf16, name="gkp2", tag="gkp2")
            gkT2 = sb.tile([B, 2 * B], bf16, name="gkT2", tag="gkT2")
            gbc = sb.tile([BLK, 2 * B], f32, name="gbc", tag="gbc")
            outw = sb.tile([BLK, 2 * B], f32, name="outw", tag="outw")
            ps_g = psp.tile([2 * B, 2 * NB], f32, name="ps_g", tag="ps_g")
            ps_w = psp.tile([BLK, 2 * B], f32, name="ps_w", tag="ps_w")
            ps_b = psp.tile([BLK, 2 * B], f32, name="ps_b", tag="ps_b")

            # constants staged while the DMAs stream (gkp2 is zeroed so the
            # later block-transposes read initialized data; only columns 0
            # and 32 carry the two gate rows)
            nc.gpsimd.memset(ones[:], 1.0)
            nc.gpsimd.memset(gkp2[:], 0.0)

            engs = [nc.sync, nc.scalar]
            for q, entries in sorted(plan.items()):
                for e in entries:
                    if e[0] == "small":
                        _, k0, k1 = e
                        engs[q].dma_start(small[:, k0:k1, :],
                                          small_d.ap()[:, k0:k1, :])
                    elif e[0] == "rhs":
                        _, k0, k1 = e
                        engs[q].dma_start(rhs[:, k0:k1, :],
                                          rhs_d.ap()[:, k0:k1, :])
                    elif e[0] == "epib":
                        engs[q].dma_start(epib[:], epib_d.ap())
                    else:
                        raise ValueError(e)

            # gate linear: M=64 ([x_hi|x_lo]) x N=32 ([gw_hi|gw_lo]); tile 16
            # carries (ones-row, gate_b/3) so each of the four PSUM quadrants
            # picks up gate_b/3 and the 3-quadrant sum reconstructs +gate_b
            for t in range(KTG):
                nc.tensor.matmul(
                    ps_g[:], small[:, t, 0:64], small[:, t, 64:NSM],
                    start=(t == 0), stop=(t == KTG - 1),
                )

            # logits = hi*hi + hi*lo + lo*hi (+gate_b folded in); chained so
            # each vector op reads at most one PSUM input
            nc.vector.tensor_copy(t1[:], ps_g[0:B, 0:NB])
            nc.vector.tensor_add(t1[:], ps_g[0:B, NB:2 * NB], t1[:])
            nc.vector.tensor_add(graw[:], ps_g[B:2 * B, 0:NB], t1[:])
            nc.scalar.activation(g[:], graw[:],
                                 mybir.ActivationFunctionType.Sigmoid)
            nc.vector.max(m8[:], g[:])
            nc.vector.match_replace(rep[:], m8[:], g[:], 0.0)
            nc.vector.tensor_sub(gk[:], g[:], rep[:])

            # stage both gate rows at partition 0: one strided copy puts
            # gk[:,0] in column 0 and gk[:,1] in column 32, then a single
            # 2-block DVE transpose lands them as row 0 of each 32-block;
            # bf16 so the downstream K=1 broadcast matmul runs single-pass
            nc.vector.tensor_copy(gkp2[:, 0:2 * B:B], gk[:, 0:2])
            nc.vector.transpose(gkT2[:], gkp2[:])

            # main matmul: W k-tile stationary (M=128), x_hi moving (N=32);
            # one psum tile holds both blocks' out.T in disjoint column
            # halves.  The gate-broadcast K=1 matmul (N=64, both blocks at
            # once) is slotted before the last k-tile so only the final W
            # segment gates the epilogue.
            for t in range(KT):
                if t == KT - 1:
                    nc.tensor.matmul(ps_b[:], ones[0:1, :], gkT2[0:1, :],
                                     start=True, stop=True)
                for h in range(2):
                    # start resets the ENTIRE psum bank, so only the very
                    # first matmul sets it (h=1 accumulates into the half
                    # the h=0 start just zeroed)
                    nc.tensor.matmul(
                        ps_w[:, h * B:(h + 1) * B],
                        rhs[:, t, h * BLK:(h + 1) * BLK],
                        small[:, t, 0:B],
                        start=(t == 0 and h == 0), stop=(t == KT - 1),
                        skip_group_check=True,
                    )

            # out.T = ps_w * gbc + bias: one gbc copy + one multiply over
            # both blocks, then the per-block bias adds run on DVE and
            # GpSimd concurrently and each half's store issues when ready
            nc.vector.tensor_copy(gbc[:], ps_b[:])
            nc.vector.tensor_mul(outw[:], ps_w[:], gbc[:])
            for h, eng in ((0, nc.vector), (1, nc.gpsimd)):
                sl = slice(h * B, (h + 1) * B)
                eng.tensor_scalar_add(outw[:, sl], outw[:, sl],
                                      epib[:, h:h + 1])
                engs[(h + 1) % 2].dma_start(out_d.ap()[:, sl], outw[:, sl])

    nc.compile()
    return nc


def get_nc(plan=None):
    plan = plan if plan is not None else DEFAULT_PLAN
    key = repr(sorted(plan.items()))
    if key not in _compiled:
        _compiled[key] = _build(plan)
    return _compiled[key]


def _tile_major(a):
    """(D, n) -> (128, KT, n) k-tile-major contiguous."""
    n = a.shape[1]
    return np.ascontiguousarray(a.reshape(KT, 128, n).transpose(1, 0, 2))


def _hi_lo(a):
    import ml_dtypes
    hi = a.astype(ml_dtypes.bfloat16)
    lo = (a - hi.astype(np.float32)).astype(ml_dtypes.bfloat16)
    return hi, lo


def build_in_maps(x, gate_w, gate_b, weight, bias):
    import ml_dtypes

    x = np.asarray(x, dtype=np.float32)
    gate_w = np.asarray(gate_w, dtype=np.float32)
    gate_b = np.asarray(gate_b, dtype=np.float32)
    weight = np.asarray(weight, dtype=np.float32)
    bias = np.asarray(bias, dtype=np.float32)

    x_hi, x_lo = _hi_lo(np.ascontiguousarray(x.T))               # (2048, 32)
    in_maps = []
    for c in range(N_CORES):
        perm = [2 * c, 2 * c + 1] + [k for k in range(NB)
                                     if k not in (2 * c, 2 * c + 1)]
        gw_hi, gw_lo = _hi_lo(gate_w[:, perm])                   # (2048, 16)
        small = np.concatenate([x_hi, x_lo, gw_hi, gw_lo], axis=1)  # (2048, 96)
        # gate-bias tile: row 0 has ones in the x columns and gate_b/3 in
        # both gw column halves, so the 3-quadrant logit sum picks up +gate_b
        gbt = np.zeros((BLK, NSM), dtype=small.dtype)
        gbt[0, 0:64] = 1.0
        gbt[0, 64:80] = (gate_b[perm] / 3.0).astype(small.dtype)
        gbt[0, 80:96] = gbt[0, 64:80]
        small = np.concatenate([small, gbt], axis=0)             # (2176, 96)
        w_shard = np.ascontiguousarray(weight[c * NOUT:(c + 1) * NOUT, :].T)
        bs = bias[c * NOUT:(c + 1) * NOUT]
        in_maps.append({
            "small": np.ascontiguousarray(
                small.reshape(KTG, 128, NSM).transpose(1, 0, 2)),
            "rhs": _tile_major(w_shard.astype(ml_dtypes.bfloat16)),
            "epib": np.ascontiguousarray(
                np.stack([bs[0:BLK], bs[BLK:NOUT]], axis=1).astype(np.float32)),
        })
    return in_maps


def assemble_out(parts):
    """Each part is out.T as [128 i, 2*32 (blk, b)] -> full (B, D)."""
    cols = []
    for arr in parts:
        a = np.asarray(arr).reshape(BLK, 2, B)                # (i, blk, b)
        cols.append(a.transpose(2, 1, 0).reshape(B, NOUT))    # (b, blk*128+i)
    return np.concatenate(cols, axis=1).astype(np.float32)


def _ensure_ntff_hook():
    """If a caller sets BASS_TRACE, run_bass_kernel_spmd imports
    antenv.axon_hooks, which is missing in this image; provide a working
    ctypes-backed stub so tracing degrades gracefully instead of raising."""
    try:
        from antenv.axon_hooks import get_axon_ntff_profile_hook  # noqa: F401
        return
    except ImportError:
        pass
    import contextlib
    import ctypes
    import types

    try:
        lib = ctypes.CDLL("/opt/axon/libaxon_pjrt.so")
        assert hasattr(lib, "axon_start_nrt_profile")
        lib.axon_start_nrt_profile.argtypes = [
            ctypes.POINTER(ctypes.c_int64), ctypes.c_size_t]
        lib.axon_start_nrt_profile.restype = ctypes.c_int64
        lib.axon_stop_nrt_profile.argtypes = [ctypes.c_char_p]
        lib.axon_stop_nrt_profile.restype = ctypes.c_int64

        @contextlib.contextmanager
        def _hook(output_dir, device_ids):
            import jax
            jax.devices()
            if device_ids:
                ids = (ctypes.c_int64 * len(device_ids))(*device_ids)
                rc = lib.axon_start_nrt_profile(ids, len(device_ids))
            else:
                rc = lib.axon_start_nrt_profile(None, 0)
            if rc != 0:
                raise RuntimeError(f"axon_start_nrt_profile rc={rc}")
            try:
                yield
            finally:
                lib.axon_stop_nrt_profile(str(output_dir).encode())

        hook = _hook
    except Exception:
        hook = None

    mod = types.ModuleType("antenv.axon_hooks")
    mod.get_axon_ntff_profile_hook = lambda: hook
    mod.set_axon_ntff_profile_hook = lambda h: None
    sys.modules["antenv.axon_hooks"] = mod


def kernel(x, gate_w, gate_b, weight, bias):
    _ensure_ntff_hook()
    from concourse.bass_utils import run_bass_kernel_spmd

    nc = get_nc()
    in_maps = build_in_maps(x, gate_w, gate_b, weight, bias)
    res = run_bass_kernel_spmd(nc, in_maps, list(range(N_CORES)))
    return assemble_out([res.results[c]["out"] for c in range(N_CORES)])


# revision 45
# speedup vs baseline: 1.1422x; 1.1422x over previous
"""Trainium2 Bass kernel for nn_GatedBlock (moe_routing).

Math (reference collapses): the (NB,BS,BS) reshape of weight maps block k to
rows [128k, 128k+128) of weight, so
    out[b, i] = g[b, i // 128] * (x @ W.T)[b, i] + bias[i]
with g = sigmoid(x @ gate_w + gate_b), bottom-8 of 16 gates zeroed per row.

Sharding: output-dim (i) split 8 ways -> 256 rows of W (= 2 gate blocks) per
core.  Per-core inputs (k-tile-major, partition-contiguous rows):
  small (128, KT, 96) bf16  [x_hi | x_lo | gw_hi | gw_lo]
  rhs   (128, KT, 256) bf16 W_shard.T
  epi   (32, 16) f32        gate_b[perm] broadcast over batch
  epib  (128, 2) f32        bias per output block (partition = i)

Design notes (from trace analysis):
* Main matmul runs W-STATIONARY (lhsT = W k-tile [128,128], moving = x_hi
  [128,32] -> psum holds out.T).  LDWEIGHTS ingests the stationary at ~4
  cols/cycle, so a (LDW, MM) pair takes ~27ns vs ~213ns for the x-stationary
  form — W enters the PE 4x faster.  Output leaves transposed; the host
  un-transposes (32KB, trivial).
* Gate logits use an exact bf16 hi/lo split (x@gw to ~1e-5; bf16 products
  are exact, fp32 PSUM accumulate, only the x_lo*gw_lo term is dropped).
  Top-8 RANKING is done on these logits (monotonicity of sigmoid); plain
  bf16 would flip the selection (min margin 3.4e-4) which is catastrophic.
* Sigmoid VALUE comes from a degree-13 odd polynomial on the DVE (3e-4 abs
  err on the logit range).  This keeps the scalar engine activation-free:
  ACT_TABLE_LOADs were observed to stall the scalar HWDGE queue ~1.4us.
* Gating in the transposed orientation: gk rows are partition-broadcast via
  K=1 ones-matmuls (DVE 32x32 block-transposes put each block's gate row at
  partition 0 first).  Epilogue block 0 runs on DVE, block 1 on GpSimd.
* W in bf16 halves the dominant DMA (1.9e-3 rel err vs the 2e-2 gate).  The
  two HWDGE queues sustain ~130-150GB/s each concurrently; bytes are split
  so both queues finish together, with W arrival order matching the main
  matmuls' k-order consumption.
"""

import sys

for _p in ("/opt/trn_rl_repo", "/root/.axon_site/_ro/trn_rl_repo"):
    if _p not in sys.path:
        sys.path.append(_p)

import numpy as np

B = 32          # batch
D = 2048        # model dim
NB = 16         # gate blocks
BLK = D // NB   # 128 output rows per gate block
N_CORES = 8
NOUT = D // N_CORES       # 256 output cols per core
KT = D // 128             # 16 k-tiles
KTG = KT + 1              # gate k-tiles: +1 tile carrying the gate bias
NSM = 96                  # small cols: 64 x (hi|lo) + 32 gw (hi|lo)

# DMA plan: per queue (sync=0, scalar=1), ordered entries.  The scalar
# queue's start is taxed by the sigmoid ACT_TABLE fetches, so it carries
# only W; the k-ranges are ordered to match the main matmuls' consumption.
DEFAULT_PLAN = {
    0: [("small", 0, KTG), ("epib",), ("rhs", 10, 16)],
    1: [("rhs", 0, 5), ("rhs", 5, 10)],
}

_compiled = {}


def _build(plan):
    import concourse.bacc as bacc
    import concourse.tile as tile
    import concourse.mybir as mybir

    f32 = mybir.dt.float32
    bf16 = mybir.dt.bfloat16
    Alu = mybir.AluOpType

    nc = bacc.Bacc("TRN2", target_bir_lowering=False, debug=False,
                   num_devices=N_CORES)

    small_d = nc.dram_tensor("small", [128, KTG, NSM], bf16, kind="ExternalInput")
    rhs_d = nc.dram_tensor("rhs", [128, KT, NOUT], bf16, kind="ExternalInput")
    epib_d = nc.dram_tensor("epib", [BLK, 2], f32, kind="ExternalInput")
    out_d = nc.dram_tensor("out", [BLK, 2 * B], f32, kind="ExternalOutput")

    with tile.TileContext(nc) as tc:
        with (
            tc.tile_pool(name="sb", bufs=1) as sb,
            tc.tile_pool(name="ps", bufs=1, space="PSUM") as psp,
        ):
            small = sb.tile([128, KTG, NSM], bf16, name="small_sb", tag="small_sb")
            rhs = sb.tile([128, KT, NOUT], bf16, name="rhs_sb", tag="rhs_sb")
            epib = sb.tile([BLK, 2], f32, name="epib_sb", tag="epib_sb")
            t1 = sb.tile([B, NB], f32, name="t1", tag="t1")
            graw = sb.tile([B, NB], f32, name="graw", tag="graw")
            g = sb.tile([B, NB], f32, name="g", tag="g")
            m8 = sb.tile([B, 8], f32, name="m8", tag="m8")
            rep = sb.tile([B, NB], f32, name="rep", tag="rep")
            gk = sb.tile([B, NB], f32, name="gk", tag="gk")
            ones = sb.tile([1, BLK], bf16, name="ones", tag="ones")
            gkp2 = sb.tile([B, 2 * B], bf16, name="gkp2", tag="gkp2")
            gkT2 = sb.tile([B, 2 * B], bf16, name="gkT2", tag="gkT2")
            gbc = sb.tile([BLK, 2 * B], f32, name="gbc", tag="gbc")
            outw = sb.tile([BLK, 2 * B], f32, name="outw", tag="outw")
            ps_g = psp.tile([2 * B, 2 * NB], f32, name="ps_g", tag="ps_g")
            ps_w = psp.tile([BLK, 2 * B], f32, name="ps_w", tag="ps_w")
            ps_b = psp.tile([BLK, 2 * B], f32, name="ps_b", tag="ps_b")

            # constants staged while the DMAs stream (gkp2 is zeroed so the
            # later block-transposes read initialized data; only columns 0
            # and 32 carry the two gate rows)
            nc.gpsimd.memset(ones[:], 1.0)
            nc.gpsimd.memset(gkp2[:], 0.0)

            engs = [nc.sync, nc.scalar]
            for q, entries in sorted(plan.items()):
                for e in entries:
                    if e[0] == "small":
                        _, k0, k1 = e
                        engs[q].dma_start(small[:, k0:k1, :],
                                          small_d.ap()[:, k0:k1, :])
                    elif e[0] == "rhs":
                        _, k0, k1 = e
                        engs[q].dma_start(rhs[:, k0:k1, :],
                                          rhs_d.ap()[:, k0:k1, :])
                    elif e[0] == "epib":
                        engs[q].dma_start(epib[:], epib_d.ap())
                    else:
                        raise ValueError(e)

            # gate linear: M=64 ([x_hi|x_lo]) x N=32 ([gw_hi|gw_lo]); tile 16
            # carries (ones-row, gate_b/3) so each of the four PSUM quadrants
            # picks up gate_b/3 and the 3-quadrant sum reconstructs +gate_b
            for t in range(KTG):
                nc.tensor.matmul(
                    ps_g[:], small[:, t, 0:64], small[:, t, 64:NSM],
                    start=(t == 0), stop=(t == KTG - 1),
                )

            # logits = hi*hi + hi*lo + lo*hi (+gate_b folded in); chained so
            # each vector op reads at most one PSUM input
            nc.vector.tensor_copy(t1[:], ps_g[0:B, 0:NB])
            nc.vector.tensor_add(t1[:], ps_g[0:B, NB:2 * NB], t1[:])
            nc.vector.tensor_add(graw[:], ps_g[B:2 * B, 0:NB], t1[:])
            nc.scalar.activation(g[:], graw[:],
                                 mybir.ActivationFunctionType.Sigmoid)
            nc.vector.max(m8[:], g[:])
            nc.vector.match_replace(rep[:], m8[:], g[:], 0.0)
            nc.vector.tensor_sub(gk[:], g[:], rep[:])

            # stage both gate rows at partition 0: one strided copy puts
            # gk[:,0] in column 0 and gk[:,1] in column 32, then a single
            # 2-block DVE transpose lands them as row 0 of each 32-block;
            # bf16 so the downstream K=1 broadcast matmul runs single-pass
            nc.vector.tensor_copy(gkp2[:, 0:2 * B:B], gk[:, 0:2])
            nc.vector.transpose(gkT2[:], gkp2[:])

            # main matmul: W k-tile stationary (M=128), x_hi moving (N=32);
            # one psum tile holds both blocks' out.T in disjoint column
            # halves.  The gate-broadcast K=1 matmul (N=64, both blocks at
            # once) is slotted before the last k-tile so only the final W
            # segment gates the epilogue.
            for t in range(KT):
                if t == KT - 1:
                    nc.tensor.matmul(ps_b[:], ones[0:1, :], gkT2[0:1, :],
                                     start=True, stop=True)
                for h in range(2):
                    # start resets the ENTIRE psum bank, so only the very
                    # first matmul sets it (h=1 accumulates into the half
                    # the h=0 start just zeroed)
                    nc.tensor.matmul(
                        ps_w[:, h * B:(h + 1) * B],
                        rhs[:, t, h * BLK:(h + 1) * BLK],
                        small[:, t, 0:B],
                        start=(t == 0 and h == 0), stop=(t == KT - 1),
                        skip_group_check=True,
                    )

            # out.T = ps_w * gbc + bias: one gbc copy + one multiply over
            # both blocks, then the per-block bias adds run on DVE and
            # GpSimd concurrently and each half's store issues when ready
            nc.vector.tensor_copy(gbc[:], ps_b[:])
            nc.vector.tensor_mul(outw[:], ps_w[:], gbc[:])
            for h, eng in ((0, nc.vector), (1, nc.gpsimd)):
                sl = slice(h * B, (h + 1) * B)
                eng.tensor_scalar_add(outw[:, sl], outw[:, sl],
                                      epib[:, h:h + 1])
                engs[(h + 1) % 2].dma_start(out_d.ap()[:, sl], outw[:, sl])

    nc.compile()
    return nc


def get_nc(plan=None):
    plan = plan if plan is not None else DEFAULT_PLAN
    key = repr(sorted(plan.items()))
    if key not in _compiled:
        _compiled[key] = _build(plan)
    return _compiled[key]


def _tile_major(a):
    """(D, n) -> (128, KT, n) k-tile-major contiguous."""
    n = a.shape[1]
    return np.ascontiguousarray(a.reshape(KT, 128, n).transpose(1, 0, 2))


def _hi_lo(a):
    import ml_dtypes
    hi = a.astype(ml_dtypes.bfloat16)
    lo = (a - hi.astype(np.float32)).astype(ml_dtypes.bfloat16)
    return hi, lo


def build_in_maps(x, gate_w, gate_b, weight, bias):
    import ml_dtypes

    x = np.asarray(x, dtype=np.float32)
    gate_w = np.asarray(gate_w, dtype=np.float32)
    gate_b = np.asarray(gate_b, dtype=np.float32)
    weight = np.asarray(weight, dtype=np.float32)
    bias = np.asarray(bias, dtype=np.float32)

    x_hi, x_lo = _hi_lo(np.ascontiguousarray(x.T))               # (2048, 32)
    in_maps = []
    for c in range(N_CORES):
        perm = [2 * c, 2 * c + 1] + [k for k in range(NB)
                                     if k not in (2 * c, 2 * c + 1)]
        gw_hi, gw_lo = _hi_lo(gate_w[:, perm])                   # (2048, 16)
        small = np.concatenate([x_hi, x_lo, gw_hi, gw_lo], axis=1)  # (2048, 96)
        # gate-bias tile: row 0 has ones in the x columns and gate_b/3 in
        # both gw column halves, so the 3-quadrant logit sum picks up +gate_b
        gbt = np.zeros((BLK, NSM), dtype=small.dtype)
        gbt[0, 0:64] = 1.0
        gbt[0, 64:80] = (gate_b[perm] / 3.0).astype(small.dtype)
        gbt[0, 80:96] = gbt[0, 64:80]
        small = np.concatenate([small, gbt], axis=0)             # (2176, 96)
        w_shard = np.ascontiguousarray(weight[c * NOUT:(c + 1) * NOUT, :].T)
        bs = bias[c * NOUT:(c + 1) * NOUT]
        in_maps.append({
            "small": np.ascontiguousarray(
                small.reshape(KTG, 128, NSM).transpose(1, 0, 2)),
            "rhs": _tile_major(w_shard.astype(ml_dtypes.bfloat16)),
            "epib": np.ascontiguousarray(
                np.stack([bs[0:BLK], bs[BLK:NOUT]], axis=1).astype(np.float32)),
        })
    return in_maps


def assemble_out(parts):
    """Each part is out.T as [128 i, 2*32 (blk, b)] -> full (B, D)."""
    cols = []
    for arr in parts:
        a = np.asarray(arr).reshape(BLK, 2, B)                # (i, blk, b)
        cols.append(a.transpose(2, 1, 0).reshape(B, NOUT))    # (b, blk*128+i)
    return np.concatenate(cols, axis=1).astype(np.float32)


def _ensure_ntff_hook():
    """If a caller sets BASS_TRACE, run_bass_kernel_spmd imports
    antenv.axon_hooks, which is missing in this image; provide a working
    ctypes-backed stub so tracing degrades gracefully instead of raising."""
    try:
        from antenv.axon_hooks import get_axon_ntff_profile_hook  # noqa: F401
        return
    except ImportError:
        pass
    import contextlib
    import ctypes
    import types

    try:
        lib = ctypes.CDLL("/opt/axon/libaxon_pjrt.so")
        assert hasattr(lib, "axon_start_nrt_profile")
        lib.axon_start_nrt_profile.argtypes = [
            ctypes.POINTER(ctypes.c_int64), ctypes.c_size_t]
        lib.axon_start_nrt_profile.restype = ctypes.c_int64
        lib.axon_stop_nrt_profile.argtypes = [ctypes.c_char_p]
        lib.axon_stop_nrt_profile.restype = ctypes.c_int64

        @contextlib.contextmanager
        def _hook(output_dir, device_ids):
            import jax
            jax.devices()
            if device_ids:
                ids = (ctypes.c_int64 * len(device_ids))(*device_ids)
                rc = lib.axon_start_nrt_profile(ids, len(device_ids))
            else:
                rc = lib.axon_start_nrt_profile(None, 0)
            if rc != 0:
                raise RuntimeError(f"axon_start_nrt_profile rc={rc}")
            try:
                yield
            finally:
                lib.axon_stop_nrt_profile(str(output_dir).encode())

        hook = _hook
    except Exception:
        hook = None

    mod = types.ModuleType("antenv.axon_hooks")
    mod.get_axon_ntff_profile_hook = lambda: hook
    mod.set_axon_ntff_profile_hook = lambda h: None
    sys.modules["antenv.axon_hooks"] = mod


def kernel(x, gate_w, gate_b, weight, bias):
    _ensure_ntff_hook()
    from concourse.bass_utils import run_bass_kernel_spmd

    nc = get_nc()
    in_maps = build_in_maps(x, gate_w, gate_b, weight, bias)
    res = run_bass_kernel_spmd(nc, in_maps, list(range(N_CORES)))
    return assemble_out([res.results[c]["out"] for c in range(N_CORES)])
